# revision 7
# baseline (speedup 1.0000x reference)
"""Causal dot-product attention, B=16 heads sharded 2-per-core across 8 TRN2 cores.

v3 architecture — all data-layout work on the HOST; the device does only the
irreducible compute; every engine stream is kept ~independent so the in-order
queues never chain across engines:

  HOST pre:  qT,kT = q,k transposed to [d=128, seq] fp16 (no PE transposes /
             DVE copies on device); v pre-tiled to [128, 16, 128] fp16.
  DEVICE, per head, per 512-wide q-block c (chunk pairs = groups; diagonal
  k-tiles first, trimmed to live columns):
    sT[k,q] = kT_j.T @ qT_blk   (PE fp16->PSUM f32; emitted THREE groups
                                 ahead of the consuming PV so the PE queue
                                 never head-blocks on exp/mask)
    pT = exp(sT/sqrt(D))        one instr per group: ACT exp for most, ~22%
                                of full-pair groups on DVE via Schraudolph
                                fast-exp (i16 = rint(sT*A+B) bitcast fp16)
    diagonal chunks:            narrow GPSIMD affine_select (width 128/256)
    stage[:,slot] = pT0 + pT1   (DVE pair-sum, fp16; four slots per stage
                                 tile; DMA'd out when full — denominator is
                                 finished on the HOST)
    out2[d,q] += v_j.T @ pT     (PE accumulate in PSUM)
  block end: DVE copy out2 -> fp16 SBUF (staged one group into the next
             block), DMA out.
  HOST post: den[q] = sum of staged pair-sums over the partition axis (f32),
             out = (out2 / den).T.

Engine budget per core (cost model): PE ~29.4us, ACT ~29.7, DVE ~27.5,
Pool ~11.6, DMA ~25 (vs v1 baseline PE 41/DVE 39/ACT 38 in 69.8us).

Numerics: fp16 PE path ~4e-4; Schraudolph exp (1.77% rms, multiplicative)
on ~22% of softmax mass => ~8e-3 total, well inside the 2e-2 gate (num and
den use the same p values, so the common error cancels).
"""

import numpy as np

import concourse.bass as bass
import concourse.mybir as mybir
import concourse.tile as tile
from concourse.bass import ds
from concourse.bass_utils import run_bass_kernel_spmd

N_CORES = 8
HPC = 2          # heads per core
N = 2048
D = 128
NBLK = N // 512  # 4 q-blocks
NSTG = 5         # 20 groups/head, 4 pair-sum slots per stage tile
SCALE = 1.0 / float(np.sqrt(D))
F32 = mybir.dt.float32
F16 = mybir.dt.float16
I16 = mybir.dt.int16

# Schraudolph fast-exp constants (fp16 bit pattern): exp(x) ~= bitcast(
# int16(rint(x * 1024/ln2 + (15360 - SIGMA)))).  Input is the RAW score, so
# the softmax 1/sqrt(D) is folded into the multiplier.
SCHRAUD_A = 1477.3195 * SCALE
SIGMA = 60.0
SCHRAUD_B = 15360.0 - SIGMA


def _split_excess_waits(nc, max_waits=1):
    """This walrus build rejects >1 sync-wait command on CTRL-queue
    instructions (Tile's kernel-tail drain carries one per live semaphore).
    Hoist excess waits onto preceding NoOps on the same engine."""
    import bass_rust

    ctr = 0
    for f in nc.m.functions:
        for bb in f.blocks:
            new_list = []
            changed = False
            for inst in bb.instructions:
                si = inst.sync_info
                if si is not None and si.on_wait and len(si.on_wait) > max_waits:
                    waits = list(si.on_wait)
                    extra, keep = waits[:-max_waits], waits[-max_waits:]
                    for i in range(0, len(extra), max_waits):
                        nop = bass_rust.InstNoOp(
                            name=f"I-waitsplit-{ctr}", ins=[], outs=[]
                        )
                        ctr += 1
                        nop.engine = inst.engine
                        nop.sync_info = mybir.SyncInfo(
                            on_wait=extra[i : i + max_waits], on_update=[]
                        )
                        new_list.append(nop)
                    inst.sync_info = mybir.SyncInfo(
                        on_wait=keep, on_update=list(si.on_update or [])
                    )
                    changed = True
                new_list.append(inst)
            if changed:
                bb.instructions = new_list


def _groups_of_block(c):
    """Chunk order for q-block c: diagonal k-tiles first (trimmed to their
    causally-live columns), then full k-tiles; paired into groups.
    Returns [(i0, [(j, trim, m), (j, trim, m)]), ...] where m is the
    diagonal index (None for full chunks)."""
    diag = [(4 * c + m, 128 * m, m) for m in range(4)]
    full = [(j, 0, None) for j in range(4 * c)]
    ch = diag + full
    return [(i, ch[i : i + 2]) for i in range(0, len(ch), 2)]


# Full-pair groups (c, group_index) whose exp runs on DVE via Schraudolph.
DVE_EXP = {(1, 2), (2, 2), (3, 2), (3, 4)}


def _build_attention_nc():
    nc = bass.Bass("TRN2", target_bir_lowering=False, debug=False, num_devices=N_CORES)
    qT_d = nc.dram_tensor("qT", [HPC, 128, N], F16, kind="ExternalInput")
    kT_d = nc.dram_tensor("kT", [HPC, 128, N], F16, kind="ExternalInput")
    v_d = nc.dram_tensor("v", [HPC, 128, N // 128, 128], F16, kind="ExternalInput")
    o2_d = nc.dram_tensor("o2", [HPC, NBLK, 128, 512], F16, kind="ExternalOutput")
    den_d = nc.dram_tensor("den", [HPC, NSTG, 128, 4, 512], F16, kind="ExternalOutput")

    with tile.TileContext(nc) as tc:
        with (
            tc.tile_pool(name="consts", bufs=1) as consts,
            tc.tile_pool(name="inp", bufs=2) as inp,
            tc.tile_pool(name="pt", bufs=6) as ptp,
            tc.tile_pool(name="stg", bufs=2) as stgp,
            tc.tile_pool(name="outsb", bufs=3) as outp,
            tc.tile_pool(name="ps_s", bufs=3, space="PSUM") as ps_s,
            tc.tile_pool(name="ps_o", bufs=2, space="PSUM") as ps_o,
        ):
            # PE p-state warmup: a chain of junk matmuls spanning the input
            # DMA window starts the ramp clock early, so real matmuls run at
            # full clock from the first group.
            scratch = consts.tile([128, 256], F16)
            nc.vector.memset(scratch, 0.25)
            warm = ps_s.tile([128, 2, 512], F32, tag="sT", name="warmup")
            for wi in range(8):
                nc.tensor.matmul(
                    warm[:, wi % 2, 0:256],
                    lhsT=scratch[:, 0:128],
                    rhs=scratch,
                    start=True,
                    stop=True,
                )

            # Input DMAs.  HWDGE (nc.sync) is a single shared device with
            # ~625ns per-DMA issue cost; Pool's SWDGE path is independent, so
            # head 0's v/k-rest ride on Pool to parallelize the startup.
            kc0, qc0, vc0, krest, qrest, vrest = {}, {}, {}, {}, {}, {}
            for h in range(HPC):
                hw_eng = nc.sync
                pl_eng = nc.gpsimd if h == 0 else nc.sync
                kc0[h] = inp.tile([128, 512], F16, tag="kc0", name=f"kc0_{h}")
                hw_eng.dma_start(out=kc0[h], in_=kT_d[h, :, ds(0, 512)])
                qc0[h] = inp.tile([128, 512], F16, tag="qc0", name=f"qc0_{h}")
                hw_eng.dma_start(out=qc0[h], in_=qT_d[h, :, ds(0, 512)])
                vc0[h] = inp.tile([128, 4, 128], F16, tag="vc0", name=f"vc0_{h}")
                pl_eng.dma_start(out=vc0[h], in_=v_d[h, :, ds(0, 4), :])
                krest[h] = inp.tile([128, 3, 512], F16, tag="krest", name=f"kr_{h}")
                pl_eng.dma_start(
                    out=krest[h],
                    in_=kT_d[h, :, ds(512, 1536)].rearrange("p (c w) -> p c w", c=3),
                )
                qrest[h] = inp.tile([128, 3, 512], F16, tag="qrest", name=f"qr_{h}")
                hw_eng.dma_start(
                    out=qrest[h],
                    in_=qT_d[h, :, ds(512, 1536)].rearrange("p (c w) -> p c w", c=3),
                )
                vrest[h] = inp.tile([128, 12, 128], F16, tag="vrest", name=f"vr_{h}")
                pl_eng.dma_start(out=vrest[h], in_=v_d[h, :, ds(4, 12), :])

            def kt_view(h, j):
                if j < 4:
                    return kc0[h][:, ds(128 * j, 128)]
                return krest[h][:, j // 4 - 1, ds(128 * (j % 4), 128)]

            def qt_view(h, c, lo, w):
                if c == 0:
                    return qc0[h][:, ds(lo, w)]
                return qrest[h][:, c - 1, ds(lo, w)]

            def v_view(h, j):
                if j < 4:
                    return vc0[h][:, j]
                return vrest[h][:, j - 4]

            groups = []
            for h in range(HPC):
                for c in range(NBLK):
                    for gib, (i0, pair) in enumerate(_groups_of_block(c)):
                        groups.append((h, c, gib, i0, pair))

            sT_of = {}

            def emit_s(gi):
                h, c, gib, i0, pair = groups[gi]
                sT = ps_s.tile([128, 2, 512], F32, tag="sT", name=f"sT_{gi}")
                for jj, (j, trim, _m) in enumerate(pair):
                    nc.tensor.matmul(
                        sT[:, jj, ds(trim, 512 - trim)],
                        lhsT=kt_view(h, j),
                        rhs=qt_view(h, c, trim, 512 - trim),
                        start=True,
                        stop=True,
                    )
                sT_of[gi] = sT

            o2sb_pend = None  # (h, c, out2) awaiting fp16 copy + DMA
            out2 = None
            stage = None

            for gi in range(min(3, len(groups))):
                emit_s(gi)
            for gi, (h, c, gib, i0, pair) in enumerate(groups):
                if gi + 3 < len(groups):
                    emit_s(gi + 3)
                gidx = gi % 20          # group index within this head
                nch = 4 * c + 4
                if i0 == 0:
                    out2 = ps_o.tile([128, 512], F32, tag="o2", name=f"o2_{h}_{c}")
                # previous block's out2 copy first in this iteration's DVE
                # stream (its dependency is the oldest)
                if o2sb_pend is not None:
                    ph, pc, pout2 = o2sb_pend
                    o2sb = outp.tile([128, 512], F16, tag="o2sb")
                    nc.vector.tensor_copy(o2sb, pout2)
                    nc.sync.dma_start(out=o2_d[ph, pc], in_=o2sb)
                    o2sb_pend = None
                sT = sT_of.pop(gi)
                pT = ptp.tile([128, 2, 512], F16, tag="pT", name=f"pT_{gi}")
                trim0 = pair[0][1]
                w = 512 - trim0
                if (c, gib) in DVE_EXP:
                    # Schraudolph fast-exp on DVE (full pairs only)
                    nc.vector.tensor_scalar(
                        out=pT[:, :, ds(trim0, w)].bitcast(I16),
                        in0=sT[:, :, ds(trim0, w)],
                        scalar1=SCHRAUD_A,
                        scalar2=SCHRAUD_B,
                        op0=mybir.AluOpType.mult,
                        op1=mybir.AluOpType.add,
                    )
                else:
                    nc.scalar.activation(
                        out=pT[:, :, ds(trim0, w)],
                        in_=sT[:, :, ds(trim0, w)],
                        func=mybir.ActivationFunctionType.Exp,
                        scale=SCALE,
                    )
                for jj, (j, trim, m) in enumerate(pair):
                    if m is not None:
                        # causal mask, narrowed to the partially-live columns
                        mw = 128 * (m + 1) - trim0
                        nc.gpsimd.affine_select(
                            out=pT[:, jj, ds(trim0, mw)],
                            in_=pT[:, jj, ds(trim0, mw)],
                            compare_op=mybir.AluOpType.is_ge,
                            fill=0.0,
                            base=trim0 - 128 * m,
                            pattern=[[1, mw]],
                            channel_multiplier=-1,
                        )
                # denominator pair-sum into the stage tile (host finishes the
                # reduction; garbage below trim0 is sliced off on the host)
                if gidx % 4 == 0:
                    stage = stgp.tile([128, 4, 512], F16, tag="stg", name=f"stg_{gi}")
                nc.vector.tensor_tensor(
                    out=stage[:, gidx % 4, ds(trim0, w)],
                    in0=pT[:, 0, ds(trim0, w)],
                    in1=pT[:, 1, ds(trim0, w)],
                    op=mybir.AluOpType.add,
                )
                if gidx == 18:
                    # shorten the kernel tail: ship the last stage's first
                    # three slots early, slot 3 alone right after
                    nc.gpsimd.dma_start(
                        out=den_d[h, 4, :, ds(0, 3), :], in_=stage[:, ds(0, 3), :]
                    )
                elif gidx == 19:
                    nc.gpsimd.dma_start(
                        out=den_d[h, 4, :, ds(3, 1), :], in_=stage[:, ds(3, 1), :]
                    )
                elif gidx % 4 == 3:
                    # alternate den-stage stores between the Pool SWDGE path
                    # and HWDGE to keep both below saturation
                    eng = nc.gpsimd if (gidx // 4) % 2 == 0 else nc.sync
                    eng.dma_start(out=den_d[h, gidx // 4], in_=stage)
                for jj, (j, trim, m) in enumerate(pair):
                    nc.tensor.matmul(
                        out2[:, ds(trim, 512 - trim)],
                        lhsT=v_view(h, j),
                        rhs=pT[:, jj, ds(trim, 512 - trim)],
                        start=(i0 == 0 and jj == 0),
                        stop=(i0 + jj == nch - 1),
                        skip_group_check=True,
                    )
                if i0 + 2 >= nch:
                    if gi == len(groups) - 1:
                        # stream-final tail: split the copy AND the store so
                        # each DMA starts as soon as its half is staged
                        o2sb = outp.tile([128, 512], F16, tag="o2sb")
                        for hf in range(2):
                            nc.vector.tensor_copy(
                                o2sb[:, ds(256 * hf, 256)],
                                out2[:, ds(256 * hf, 256)],
                            )
                            nc.sync.dma_start(
                                out=o2_d[h, c, :, ds(256 * hf, 256)],
                                in_=o2sb[:, ds(256 * hf, 256)],
                            )
                    else:
                        o2sb_pend = (h, c, out2)

    _split_excess_waits(nc)
    return nc


_NC_CACHE = []


def kernel(q: np.ndarray, k: np.ndarray, v: np.ndarray) -> np.ndarray:
    assert q.shape == (N_CORES * HPC, N, D)
    if not _NC_CACHE:
        _NC_CACHE.append(_build_attention_nc())
    nc = _NC_CACHE[0]
    q16 = q.astype(np.float16)
    k16 = k.astype(np.float16)
    v16 = v.astype(np.float16)
    in_maps = []
    for i in range(N_CORES):
        sl = slice(HPC * i, HPC * (i + 1))
        qT = np.ascontiguousarray(q16[sl].transpose(0, 2, 1))
        kT = np.ascontiguousarray(k16[sl].transpose(0, 2, 1))
        vt = np.ascontiguousarray(
            v16[sl].reshape(HPC, N // 128, 128, D).transpose(0, 2, 1, 3)
        )
        in_maps.append({"qT": qT, "kT": kT, "v": vt})
    last_err = None
    for _attempt in range(4):
        try:
            res = run_bass_kernel_spmd(nc, in_maps, list(range(N_CORES)))
            break
        except Exception as e:  # transient device wedge: reset backend, retry
            last_err = e
            try:
                import jax

                jax.clear_caches()
                jax.extend.backend.clear_backends()
            except Exception:
                pass
            import time

            time.sleep(5)
    else:
        raise last_err

    # group layout metadata for the host-side denominator reduction
    gmeta = []
    for c in range(NBLK):
        for gib, (i0, pair) in enumerate(_groups_of_block(c)):
            gmeta.append((c, pair[0][1]))

    out = np.empty((N_CORES * HPC, N, D), dtype=np.float32)
    for i in range(N_CORES):
        o2 = res.results[i]["o2"].astype(np.float32)    # [HPC, 4, 128, 512]
        dstg = res.results[i]["den"]                    # [HPC, 5, 128, 4, 512] f16
        for hh in range(HPC):
            den = np.zeros((NBLK, 512), dtype=np.float32)
            for gidx, (c, trim0) in enumerate(gmeta):
                sl = dstg[hh, gidx // 4, :, gidx % 4, trim0:]
                den[c, trim0:] += sl.astype(np.float32).sum(axis=0)
            o = o2[hh].transpose(0, 2, 1) / den[:, :, None]
            out[HPC * i + hh] = o.reshape(N, D)
    return out


# revision 27
# speedup vs baseline: 1.0218x; 1.0218x over previous
"""Causal dot-product attention, B=16 heads sharded 2-per-core across 8 TRN2 cores.

v3 architecture — all data-layout work on the HOST; the device does only the
irreducible compute; every engine stream is kept ~independent so the in-order
queues never chain across engines:

  HOST pre:  qT,kT = q,k transposed to [d=128, seq] fp16 (no PE transposes /
             DVE copies on device); v pre-tiled to [128, 16, 128] fp16.
  DEVICE, per head, per 512-wide q-block c (chunk pairs = groups; diagonal
  k-tiles first, trimmed to live columns):
    sT[k,q] = kT_j.T @ qT_blk   (PE fp16->PSUM f32; emitted THREE groups
                                 ahead of the consuming PV so the PE queue
                                 never head-blocks on exp/mask)
    pT = exp(sT/sqrt(D))        one instr per group: ACT exp for most, ~22%
                                of full-pair groups on DVE via Schraudolph
                                fast-exp (i16 = rint(sT*A+B) bitcast fp16)
    diagonal chunks:            narrow GPSIMD affine_select (width 128/256)
    stage[:,slot] = pT0 + pT1   (DVE pair-sum, fp16; four slots per stage
                                 tile; DMA'd out when full — denominator is
                                 finished on the HOST)
    out2[d,q] += v_j.T @ pT     (PE accumulate in PSUM)
  block end: DVE copy out2 -> fp16 SBUF (staged one group into the next
             block), DMA out.
  HOST post: den[q] = sum of staged pair-sums over the partition axis (f32),
             out = (out2 / den).T.

Engine budget per core (cost model): PE ~29.4us, ACT ~29.7, DVE ~27.5,
Pool ~11.6, DMA ~25 (vs v1 baseline PE 41/DVE 39/ACT 38 in 69.8us).

Numerics: fp16 PE path ~4e-4; Schraudolph exp (1.77% rms, multiplicative)
on ~22% of softmax mass => ~8e-3 total, well inside the 2e-2 gate (num and
den use the same p values, so the common error cancels).
"""

import numpy as np

import concourse.bass as bass
import concourse.mybir as mybir
import concourse.tile as tile
from concourse.bass import ds
from concourse.bass_utils import run_bass_kernel_spmd

N_CORES = 8
HPC = 2          # heads per core
N = 2048
D = 128
NBLK = N // 512  # 4 q-blocks
NSTG = 5         # 20 groups/head, 4 pair-sum slots per stage tile
SCALE = 1.0 / float(np.sqrt(D))
F32 = mybir.dt.float32
F16 = mybir.dt.float16
I16 = mybir.dt.int16

# Schraudolph fast-exp constants (fp16 bit pattern): exp(x) ~= bitcast(
# int16(rint(x * 1024/ln2 + (15360 - SIGMA)))).  Input is the RAW score, so
# the softmax 1/sqrt(D) is folded into the multiplier.
SCHRAUD_A = 1477.3195 * SCALE
SIGMA = 60.0
SCHRAUD_B = 15360.0 - SIGMA


def _split_excess_waits(nc, max_waits=1):
    """This walrus build rejects >1 sync-wait command on CTRL-queue
    instructions (Tile's kernel-tail drain carries one per live semaphore).
    Hoist excess waits onto preceding NoOps on the same engine."""
    import bass_rust

    ctr = 0
    for f in nc.m.functions:
        for bb in f.blocks:
            new_list = []
            changed = False
            for inst in bb.instructions:
                si = inst.sync_info
                if si is not None and si.on_wait and len(si.on_wait) > max_waits:
                    waits = list(si.on_wait)
                    extra, keep = waits[:-max_waits], waits[-max_waits:]
                    for i in range(0, len(extra), max_waits):
                        nop = bass_rust.InstNoOp(
                            name=f"I-waitsplit-{ctr}", ins=[], outs=[]
                        )
                        ctr += 1
                        nop.engine = inst.engine
                        nop.sync_info = mybir.SyncInfo(
                            on_wait=extra[i : i + max_waits], on_update=[]
                        )
                        new_list.append(nop)
                    inst.sync_info = mybir.SyncInfo(
                        on_wait=keep, on_update=list(si.on_update or [])
                    )
                    changed = True
                new_list.append(inst)
            if changed:
                bb.instructions = new_list


def _groups_of_block(c):
    """Chunk order for q-block c: full k-tiles first (their kT/v tiles are
    DMA-resident earliest), diagonal k-tiles last (trimmed to their causally
    live columns) so each block — and the kernel — drains on the cheapest
    exp/mask/PV chain.  Returns [(i0, [(j, trim, m), (j, trim, m)]), ...]
    where m is the diagonal index (None for full chunks)."""
    full = [(j, 0, None) for j in range(4 * c)]
    diag = [(4 * c + m, 128 * m, m) for m in range(4)]
    ch = full + diag
    return [(i, ch[i : i + 2]) for i in range(0, len(ch), 2)]


# Full-pair groups (c, group_index) whose exp runs on DVE via Schraudolph.
DVE_EXP = {(1, 0), (2, 1), (3, 1), (3, 3)}


def _build_attention_nc():
    nc = bass.Bass("TRN2", target_bir_lowering=False, debug=False, num_devices=N_CORES)
    # in0 = per-head concat of kT[:, :512] | qT[:, :512]: one DMA unblocks
    # the first block's compute
    in0_d = nc.dram_tensor("in0", [HPC, 128, 1024], F16, kind="ExternalInput")
    qT_d = nc.dram_tensor("qT", [HPC, 128, N], F16, kind="ExternalInput")
    kT_d = nc.dram_tensor("kT", [HPC, 128, N], F16, kind="ExternalInput")
    v_d = nc.dram_tensor("v", [HPC, 128, N // 128, 128], F16, kind="ExternalInput")
    o2_d = nc.dram_tensor("o2", [HPC, NBLK, 128, 512], F16, kind="ExternalOutput")
    den_d = nc.dram_tensor("den", [HPC, NSTG, 128, 4, 512], F16, kind="ExternalOutput")

    with tile.TileContext(nc) as tc:
        with (
            tc.tile_pool(name="consts", bufs=1) as consts,
            tc.tile_pool(name="inp", bufs=2) as inp,
            tc.tile_pool(name="pt", bufs=6) as ptp,
            tc.tile_pool(name="stg", bufs=2) as stgp,
            tc.tile_pool(name="outsb", bufs=3) as outp,
            tc.tile_pool(name="ps_s", bufs=3, space="PSUM") as ps_s,
            tc.tile_pool(name="ps_o", bufs=2, space="PSUM") as ps_o,
        ):
            # PE p-state warmup: one tiny junk matmul starts the ramp clock
            # (post-stall matmuls only pay ~2 MID-priced instructions, so a
            # long warmup chain is not worth the queue time).
            scratch = consts.tile([128, 64], F16)
            nc.vector.memset(scratch, 0.25)
            warm = ps_s.tile([128, 2, 512], F32, tag="sT", name="warmup")
            nc.tensor.matmul(
                warm[0:64, 0, 0:64], lhsT=scratch, rhs=scratch, start=True, stop=True
            )

            # Input DMAs.  HWDGE (nc.sync) is a single shared device with
            # ~625ns per-DMA issue cost; Pool's SWDGE path is independent.
            # Upfront: the urgent tensors, ordered by first use; krest_h0 and
            # qrest_h1 are emitted INSIDE the loop so they don't head-block
            # the Pool queue ahead of the first masks.
            kq0, vc0, krest, qrest, vrest = {}, {}, {}, {}, {}
            for h in range(HPC):
                kq0[h] = inp.tile([128, 1024], F16, tag="kq0", name=f"kq0_{h}")
                vc0[h] = inp.tile([128, 4, 128], F16, tag="vc0", name=f"vc0_{h}")
                krest[h] = inp.tile([128, 3, 512], F16, tag="krest", name=f"kr_{h}")
                qrest[h] = inp.tile([128, 3, 512], F16, tag="qrest", name=f"qr_{h}")
                vrest[h] = inp.tile([128, 12, 128], F16, tag="vrest", name=f"vr_{h}")
            nc.sync.dma_start(out=kq0[0], in_=in0_d[0])
            nc.sync.dma_start(out=kq0[1], in_=in0_d[1])
            nc.sync.dma_start(
                out=qrest[0],
                in_=qT_d[0, :, ds(512, 1536)].rearrange("p (c w) -> p c w", c=3),
            )
            nc.sync.dma_start(
                out=krest[1],
                in_=kT_d[1, :, ds(512, 1536)].rearrange("p (c w) -> p c w", c=3),
            )
            nc.sync.dma_start(out=vrest[1], in_=v_d[1, :, ds(4, 12), :])
            nc.gpsimd.dma_start(out=vc0[0], in_=v_d[0, :, ds(0, 4), :])
            nc.gpsimd.dma_start(out=vc0[1], in_=v_d[1, :, ds(0, 4), :])
            nc.gpsimd.dma_start(out=vrest[0], in_=v_d[0, :, ds(4, 12), :])

            def kt_view(h, j):
                if j < 4:
                    return kq0[h][:, ds(128 * j, 128)]
                return krest[h][:, j // 4 - 1, ds(128 * (j % 4), 128)]

            def qt_view(h, c, lo, w):
                # h1's q blocks are stored descending (its stream runs c3->c0)
                if h % 2 == 0:
                    if c == 0:
                        return kq0[h][:, ds(512 + lo, w)]
                    return qrest[h][:, c - 1, ds(lo, w)]
                if c == 3:
                    return kq0[h][:, ds(512 + lo, w)]
                return qrest[h][:, 2 - c, ds(lo, w)]

            def v_view(h, j):
                if j < 4:
                    return vc0[h][:, j]
                return vrest[h][:, j - 4]

            # Two interleaved streams: A = head 0 blocks ascending (ACT-heavy
            # early blocks), B = head 1 blocks DESCENDING (PE-heavy early), so
            # the PE/ACT load mix stays balanced across the whole kernel.
            def stream(h):
                order = range(NBLK) if h % 2 == 0 else range(NBLK - 1, -1, -1)
                out = []
                for c in order:
                    for gib, (i0, pair) in enumerate(_groups_of_block(c)):
                        out.append({"h": h, "c": c, "gib": gib, "i0": i0,
                                    "pair": pair, "sg": len(out)})
                return out

            A, B = stream(0), stream(1)
            seq = A[:3]
            for t in range(17):
                seq += [A[3 + t], B[t]]
            seq += B[17:]

            sT_of = {}

            def emit_s(si):
                g = seq[si]
                sT = ps_s.tile([128, 2, 512], F32, tag="sT", name=f"sT_{si}")
                for jj, (j, trim, _m) in enumerate(g["pair"]):
                    nc.tensor.matmul(
                        sT[:, jj, ds(trim, 512 - trim)],
                        lhsT=kt_view(g["h"], j),
                        rhs=qt_view(g["h"], g["c"], trim, 512 - trim),
                        start=True,
                        stop=True,
                    )
                sT_of[si] = sT

            o2sb_pend = None  # (h, c, out2) awaiting fp16 copy + DMA
            pv_pend = None    # deferred PV emission for DVE-exp groups
            out2_s = {}       # per-stream current out2 PSUM tile
            stage_s = {}      # per-stream current den stage tile

            def emit_pv(si):
                g = seq[si]
                nch = 4 * g["c"] + 4
                for jj, (j, trim, m) in enumerate(g["pair"]):
                    nc.tensor.matmul(
                        out2_of[si][:, ds(trim, 512 - trim)],
                        lhsT=v_view(g["h"], j),
                        rhs=pT_of[si][:, jj, ds(trim, 512 - trim)],
                        start=(g["i0"] == 0 and jj == 0),
                        stop=(g["i0"] + jj == nch - 1),
                        skip_group_check=True,
                    )

            out2_of = {}
            pT_of = {}
            for si in range(min(3, len(seq))):
                emit_s(si)
            for si, g in enumerate(seq):
                h, c, gib, i0, pair = g["h"], g["c"], g["gib"], g["i0"], g["pair"]
                if si + 3 < len(seq):
                    emit_s(si + 3)
                if si == 2:
                    nc.gpsimd.dma_start(
                        out=krest[0],
                        in_=kT_d[0, :, ds(512, 1536)].rearrange(
                            "p (c w) -> p c w", c=3
                        ),
                    )
                elif si == 5:
                    nc.gpsimd.dma_start(
                        out=qrest[1],
                        in_=qT_d[1, :, ds(512, 1536)].rearrange(
                            "p (c w) -> p c w", c=3
                        ),
                    )
                if pv_pend is not None:
                    # DVE-exp groups' PV is emitted one iteration late so the
                    # in-order PE queue never head-blocks on the Schraudolph
                    emit_pv(pv_pend)
                    pv_pend = None
                gidx = g["sg"]          # group index within this stream
                nch = 4 * c + 4
                if i0 == 0:
                    out2_s[h] = ps_o.tile(
                        [128, 512], F32, tag="o2", name=f"o2_{h}_{c}"
                    )
                out2_of[si] = out2_s[h]
                out2 = out2_s[h]
                # previous block's out2 copy first in this iteration's DVE
                # stream (its dependency is the oldest)
                if o2sb_pend is not None:
                    ph, pc, pout2 = o2sb_pend
                    o2sb = outp.tile([128, 512], F16, tag="o2sb")
                    nc.vector.tensor_copy(o2sb, pout2)
                    nc.sync.dma_start(out=o2_d[ph, pc], in_=o2sb)
                    o2sb_pend = None
                sT = sT_of.pop(si)
                pT = ptp.tile([128, 2, 512], F16, tag="pT", name=f"pT_{si}")
                pT_of[si] = pT
                trim0 = pair[0][1]
                w = 512 - trim0
                if (c, gib) in DVE_EXP:
                    # Schraudolph fast-exp on DVE (full pairs only)
                    nc.vector.tensor_scalar(
                        out=pT[:, :, ds(trim0, w)].bitcast(I16),
                        in0=sT[:, :, ds(trim0, w)],
                        scalar1=SCHRAUD_A,
                        scalar2=SCHRAUD_B,
                        op0=mybir.AluOpType.mult,
                        op1=mybir.AluOpType.add,
                    )
                else:
                    nc.scalar.activation(
                        out=pT[:, :, ds(trim0, w)],
                        in_=sT[:, :, ds(trim0, w)],
                        func=mybir.ActivationFunctionType.Exp,
                        scale=SCALE,
                    )
                for jj, (j, trim, m) in enumerate(pair):
                    if m is not None:
                        # causal mask, narrowed to the partially-live columns
                        mw = 128 * (m + 1) - trim0
                        nc.gpsimd.affine_select(
                            out=pT[:, jj, ds(trim0, mw)],
                            in_=pT[:, jj, ds(trim0, mw)],
                            compare_op=mybir.AluOpType.is_ge,
                            fill=0.0,
                            base=trim0 - 128 * m,
                            pattern=[[1, mw]],
                            channel_multiplier=-1,
                        )
                # denominator pair-sum into the stage tile (host finishes the
                # reduction; garbage below trim0 is sliced off on the host)
                if gidx % 4 == 0:
                    stage = stgp.tile([128, 4, 512], F16, tag="stg", name=f"stg_{gi}")
                nc.vector.tensor_tensor(
                    out=stage[:, gidx % 4, ds(trim0, w)],
                    in0=pT[:, 0, ds(trim0, w)],
                    in1=pT[:, 1, ds(trim0, w)],
                    op=mybir.AluOpType.add,
                )
                last_head = h == HPC - 1
                if last_head and gidx in (17, 18):
                    # kernel tail: fold slots 1,2 into slot 0 as they land so
                    # the final stage ships before the last group finishes
                    nc.vector.tensor_tensor(
                        out=stage[:, 0, :],
                        in0=stage[:, 0, :],
                        in1=stage[:, gidx % 4, :],
                        op=mybir.AluOpType.add,
                    )
                    if gidx == 18:
                        nc.sync.dma_start(
                            out=den_d[h, 4, :, ds(0, 1), :], in_=stage[:, ds(0, 1), :]
                        )
                elif last_head and gidx == 19:
                    # the last pair-sum ([256,512) live) ships alone, ahead
                    # of the o2 store in the HWDGE queue; host adds it in
                    nc.sync.dma_start(
                        out=den_d[h, 4, :, ds(1, 1), ds(256, 256)],
                        in_=stage[:, 3, ds(256, 256)],
                    )
                elif gidx % 4 == 3:
                    nc.sync.dma_start(out=den_d[h, gidx // 4], in_=stage)
                if (c, gib) in DVE_EXP:
                    pv_pend = gi
                else:
                    emit_pv(gi)
                if i0 + 2 >= nch:
                    if gi == len(groups) - 1:
                        o2sb = outp.tile([128, 512], F16, tag="o2sb")
                        nc.vector.tensor_copy(o2sb, out2)
                        nc.sync.dma_start(out=o2_d[h, c], in_=o2sb)
                    else:
                        o2sb_pend = (h, c, out2)

    _split_excess_waits(nc)
    return nc


_NC_CACHE = []


def kernel(q: np.ndarray, k: np.ndarray, v: np.ndarray) -> np.ndarray:
    assert q.shape == (N_CORES * HPC, N, D)
    if not _NC_CACHE:
        _NC_CACHE.append(_build_attention_nc())
    nc = _NC_CACHE[0]
    q16 = q.astype(np.float16)
    k16 = k.astype(np.float16)
    v16 = v.astype(np.float16)
    in_maps = []
    for i in range(N_CORES):
        sl = slice(HPC * i, HPC * (i + 1))
        qT = np.ascontiguousarray(q16[sl].transpose(0, 2, 1))
        kT = np.ascontiguousarray(k16[sl].transpose(0, 2, 1))
        vt = np.ascontiguousarray(
            v16[sl].reshape(HPC, N // 128, 128, D).transpose(0, 2, 1, 3)
        )
        in0 = np.ascontiguousarray(
            np.concatenate([kT[:, :, :512], qT[:, :, :512]], axis=2)
        )
        in_maps.append({"in0": in0, "qT": qT, "kT": kT, "v": vt})
    last_err = None
    for _attempt in range(4):
        try:
            res = run_bass_kernel_spmd(nc, in_maps, list(range(N_CORES)))
            break
        except Exception as e:  # transient device wedge: reset backend, retry
            last_err = e
            try:
                import jax

                jax.clear_caches()
                jax.extend.backend.clear_backends()
            except Exception:
                pass
            import time

            time.sleep(5)
    else:
        raise last_err

    # group layout metadata for the host-side denominator reduction
    gmeta = []
    for c in range(NBLK):
        for gib, (i0, pair) in enumerate(_groups_of_block(c)):
            gmeta.append((c, pair[0][1]))

    out = np.empty((N_CORES * HPC, N, D), dtype=np.float32)
    for i in range(N_CORES):
        o2 = res.results[i]["o2"].astype(np.float32)    # [HPC, 4, 128, 512]
        dstg = res.results[i]["den"]                    # [HPC, 5, 128, 4, 512] f16
        for hh in range(HPC):
            den = np.zeros((NBLK, 512), dtype=np.float32)
            for gidx, (c, trim0) in enumerate(gmeta):
                if hh == HPC - 1 and gidx >= 16:
                    continue  # stage 4 is folded into slot 0 on-device
                sl = dstg[hh, gidx // 4, :, gidx % 4, trim0:]
                den[c, trim0:] += sl.astype(np.float32).sum(axis=0)
            if hh == HPC - 1:
                # stage 4: slot 0 = groups 16..18 folded; slot 1[256:] =
                # group 19's pair-sum (all block 3)
                den[3, :] += dstg[hh, 4, :, 0, :].astype(np.float32).sum(axis=0)
                den[3, 256:] += (
                    dstg[hh, 4, :, 1, 256:].astype(np.float32).sum(axis=0)
                )
            o = o2[hh].transpose(0, 2, 1) / den[:, :, None]
            out[HPC * i + hh] = o.reshape(N, D)
    return out
